# revision 21
# baseline (speedup 1.0000x reference)
"""LocalSelfAttention (window=7) Trainium2 Bass kernel, v2.

Full inputs in, full output out. Sharding: 8 cores = batch(4) x seq-half(2),
each core handles 1024 tokens with a 3-token zero-padded halo on xs.

Math rewrites (all exact or bf16-rounding-only):
- K bias drops (softmax shift invariance, incl. zero-padded halo taps).
- V bias + FC bias fold into the residual on the host:
  xq' = x + (b_vs @ w_fc + b_fc); residual enters the FC psum via an
  identity matmul so no DVE copy/add is needed.
- 1/sqrt(dk) folds into the Q projection eviction (ACT scale).
- Band masking is a 0/1 multiply AFTER exp (garbage scores are bounded,
  exp stays finite), so scores go straight from PSUM through one wide
  ACT exp into bf16 SBUF per 8-head group.

Attention: 11 chunks of 96 queries, 102-wide key windows. Scores for 8
heads share one 2-bank PSUM tile (head slots at 128-f32 stride). Softmax
is 3 wide DVE ops per group (band-mult, reduce, normalize-mult) plus a
tiny reciprocal. Per-head PE transposes (identity matmul) and single
PV matmuls (V is projected as 11 window-aligned chunk tiles).

PSUM budget (8 banks): scores 2 + transpose-out 2 + PV-out 2 + proj/FC 2.
"""

import sys

for _p in ("/opt/trn_rl_repo",):
    if _p not in sys.path:
        sys.path.insert(0, _p)

import numpy as np
import ml_dtypes

BF16 = ml_dtypes.bfloat16

H, DK, DV, D = 16, 64, 64, 1024
NEI = 3
TEMP = 8.0
EPS = 1e-5
B, S = 4, 2048
NCORES = 8
T = (B * S) // NCORES          # 1024 tokens per core
TH = T + 2 * NEI               # 1030 halo tokens
P = 128
NT = T // P                    # 8 fc-phase token chunks
ND = D // P                    # 8 feature chunks
CL = 96                        # attention chunk length
NCH = 11                       # attention chunks
TQ = 1056                      # padded query width (NCH*CL)
KW = 1088                      # padded key block width (windows read 128)
XSW = 1152                     # padded xsT block width (V window reads)
W = 102                        # key window (CL + 2*NEI); last chunk: 96

_CACHE = {}


def _build_program(apply_affine: bool):
    import concourse.bacc as bacc
    import concourse.tile as tile
    import concourse.bass as bass
    from concourse import mybir
    from contextlib import ExitStack

    f32 = mybir.dt.float32
    bf16 = mybir.dt.bfloat16
    Alu = mybir.AluOpType
    Act = mybir.ActivationFunctionType

    from concourse import hw_specs as _hw
    _orig_tabs = _hw.get_activation_tables
    def _cov_first(arch, _o=_orig_tabs):
        t = dict(_o(arch))
        key = "natural_log_exp_and_others"
        if key in t:
            t = {key: t[key], **{k: v for k, v in t.items() if k != key}}
        return t
    _cov_first.cache_clear = getattr(_orig_tabs, "cache_clear", lambda: None)
    import os as _os
    if _os.environ.get("ACT_TABLE_PATCH", "0") == "1":
        bacc.get_activation_tables = _cov_first

    nc = bacc.Bacc(
        "TRN2", target_bir_lowering=False, debug=False, enable_asserts=False
    )

    def din(name, shape, dt_):
        return nc.dram_tensor(name, shape, dt_, kind="ExternalInput").ap()

    xqT = din("xqT", (D, T), bf16)        # x^T (host-transposed)
    xsT = din("xsT", (D, TH), bf16)       # xs^T halo (host-transposed)
    xq = din("xq", (T, D), bf16)          # x + bprime, token-major
    wq = din("wq", (ND * D, P), bf16)     # ec-major blocks: (ec,dc) -> 128x128
    wk = din("wk", (D, D), bf16)
    wv = din("wv", (D, D), bf16)
    wf = din("wf", (D, D), bf16)
    bq = din("bq", (P, ND), f32)          # b_qs/TEMP laid out [p, ec]
    band = din("band", (CL, P), bf16)     # 0/1 band mask (cols>=102 zero)
    idn = din("idn", (P, P), bf16)        # identity for PE transpose/residual
    if apply_affine:
        lng = din("lng", (1, D), f32)
        lnb = din("lnb", (1, D), f32)
    yo = nc.dram_tensor("yo", (T, D), f32, kind="ExternalOutput").ap()

    def dram_blocks_ap(src, nblk, blk_w, valid_w=None, b0=0):
        """AP over src (R, C) viewed as [128 par, nblk, valid_w] where
        block b0+b, partition p reads src row 128*(b0+b) + p, cols 0:valid_w."""
        vw = valid_w if valid_w is not None else blk_w
        c = src.ap[-1][0]  # innermost stride (elements)
        rstride = src.ap[0][0]
        return bass.AP(
            tensor=src.tensor, offset=src.offset + rstride * P * b0,
            ap=[[rstride, P], [rstride * P, nblk], [c, vw]],
        )

    with tile.TileContext(nc) as tc, ExitStack() as ctx:
        consts = ctx.enter_context(tc.tile_pool(name="consts", bufs=1))
        big = ctx.enter_context(tc.tile_pool(name="big", bufs=1))
        wpool = ctx.enter_context(tc.tile_pool(name="wpool", bufs=2))
        pepool = ctx.enter_context(tc.tile_pool(name="pepool", bufs=2))
        pnpool = ctx.enter_context(tc.tile_pool(name="pnpool", bufs=2))
        ptpool = ctx.enter_context(tc.tile_pool(name="ptpool", bufs=2))
        ypool = ctx.enter_context(tc.tile_pool(name="ypool", bufs=2))
        small = ctx.enter_context(tc.tile_pool(name="small", bufs=3))
        lns = ctx.enter_context(tc.tile_pool(name="lns", bufs=2))
        psA = ctx.enter_context(tc.tile_pool(name="psA", bufs=2, space="PSUM"))
        psS = ctx.enter_context(tc.tile_pool(name="psS", bufs=1, space="PSUM"))
        psX = ctx.enter_context(tc.tile_pool(name="psX", bufs=2, space="PSUM"))
        psF = ctx.enter_context(tc.tile_pool(name="psF", bufs=1, space="PSUM"))

        # ---- constants (vector queue; sync starts the big loads) ----
        idn_sb = consts.tile([P, P], bf16, tag="idn")
        nc.gpsimd.dma_start(out=idn_sb, in_=idn)
        band_sb = consts.tile([CL, P], bf16, tag="band")
        nc.gpsimd.dma_start(out=band_sb, in_=band)
        bq_sb = consts.tile([P, ND], f32, tag="bq")
        nc.gpsimd.dma_start(out=bq_sb, in_=bq)
        # Q-proj inputs first: xT halves + wq halves on sync/scalar
        eps_sb = consts.tile([P, 1], f32, tag="eps")
        nc.vector.memset(eps_sb, EPS)
        if apply_affine:
            g_bc = consts.tile([P, D], f32, tag="g_bc")
            b_bc = consts.tile([P, D], f32, tag="b_bc")
            nc.sync.dma_start(
                out=g_bc,
                in_=bass.AP(tensor=lng.tensor, offset=lng.offset,
                            ap=[[0, P]] + list(lng.ap[1:])),
            )
            nc.sync.dma_start(
                out=b_bc,
                in_=bass.AP(tensor=lnb.tensor, offset=lnb.offset,
                            ap=[[0, P]] + list(lnb.ap[1:])),
            )

        # ---- big loads: Q-proj inputs first, split fine to engage many
        # DMA engines (each dma_start lands on its own engine) ----
        qs = [nc.sync, nc.scalar, nc.gpsimd]
        wq_sb = wpool.tile([P, ND * D], bf16, tag="w", name="wq")
        wqv = wq_sb.rearrange("p (b c) -> p b c", b=8 * ND)   # (ec,dc) blocks
        xT_all = big.tile([P, ND * T], bf16, tag="xT")
        xtv = xT_all.rearrange("p (b c) -> p b c", b=ND)
        # xT column-halves split in 2 block-groups each; wq in ec-pair chunks:
        # Q-proj ec needs wq blocks [8*ec, 8*ec+8) and xT cols per psum half.
        for i in range(2):
            qs[i].dma_start(
                out=xtv[:, 4 * i:4 * i + 4, 0:512],
                in_=bass.AP(tensor=xqT.tensor,
                            offset=xqT.offset + 1024 * P * 4 * i * 1,
                            ap=[[1024, P], [1024 * P, 4], [1, 512]]))
        for i in range(4):
            qs[(2 + i) % 3].dma_start(
                out=wqv[:, 16 * i:16 * i + 16, :],
                in_=dram_blocks_ap(wq, 16, P, b0=16 * i))
        for i in range(2):
            qs[i].dma_start(
                out=xtv[:, 4 * i:4 * i + 4, 512:1024],
                in_=bass.AP(tensor=xqT.tensor,
                            offset=xqT.offset + 1024 * P * 4 * i + 512,
                            ap=[[1024, P], [1024 * P, 4], [1, 512]]))
        xsT_all = big.tile([P, ND * XSW], bf16, tag="xsT")
        xsv = xsT_all.rearrange("p (b c) -> p b c", b=ND)
        wk_sb = wpool.tile([P, ND * D], bf16, tag="w", name="wk")
        wkv = wk_sb.rearrange("p (b c) -> p b c", b=ND)
        for i in range(4):
            qs[i % 3].dma_start(out=xsv[:, 2 * i:2 * i + 2, 0:TH],
                                in_=dram_blocks_ap(xsT, 2, XSW, TH, b0=2 * i))
            qs[(i + 1) % 3].dma_start(out=wkv[:, 2 * i:2 * i + 2, :],
                                      in_=dram_blocks_ap(wk, 2, D, b0=2 * i))
        nc.vector.memset(xsv[:, :, TH:XSW], 0.0)

        QT = big.tile([P, ND * TQ], bf16, tag="QT")
        KT = big.tile([P, ND * KW], bf16, tag="KT")
        V_all = big.tile([P, NCH * D], bf16, tag="V")
        OT = big.tile([P, ND * T], bf16, tag="OT")

        # ---- Q projection: feature-major, bias+1/TEMP via ACT evict ----
        for ec in range(ND):
            psa = psA.tile([P, 512], f32, tag="psA", name="ps_qa")
            psb = psA.tile([P, 512], f32, tag="psA", name="ps_qb")
            for dc in range(ND):
                wqs = wq_sb[:, (ec * ND + dc) * P:(ec * ND + dc) * P + P]
                nc.tensor.matmul(psa, lhsT=wqs,
                                 rhs=xT_all[:, dc * T:dc * T + 512],
                                 start=(dc == 0), stop=(dc == ND - 1))
            for dc in range(ND):
                wqs = wq_sb[:, (ec * ND + dc) * P:(ec * ND + dc) * P + P]
                nc.tensor.matmul(psb, lhsT=wqs,
                                 rhs=xT_all[:, dc * T + 512:dc * T + 1024],
                                 start=(dc == 0), stop=(dc == ND - 1))
            nc.scalar.activation(out=QT[:, ec * TQ:ec * TQ + 512], in_=psa,
                                 func=Act.Identity,
                                 bias=bq_sb[:, ec:ec + 1], scale=1.0 / TEMP)
            nc.vector.scalar_tensor_tensor(
                out=QT[:, ec * TQ + 512:ec * TQ + 1024], in0=psb,
                scalar=1.0 / TEMP,
                in1=bq_sb[:, ec:ec + 1].to_broadcast((P, 512)),
                op0=Alu.mult, op1=Alu.add,
            )
        # zero the 32 pad query columns of each block
        qv = QT.rearrange("p (b c) -> p b c", b=ND)
        nc.vector.memset(qv[:, :, T:TQ], 0.0)

        wv_sb = wpool.tile([P, ND * D], bf16, tag="w", name="wv")
        nc.gpsimd.dma_start(
            out=wv_sb.rearrange("p (b c) -> p b c", b=ND),
            in_=dram_blocks_ap(wv, ND, D),
        )

        # ---- K projection (no bias); KT cols beyond 1030 are 0 ----
        for ec in range(ND):
            psa = psA.tile([P, 512], f32, tag="psA", name="ps_ka")
            psb = psA.tile([P, 512], f32, tag="psA", name="ps_kb")
            for dc in range(ND):
                nc.tensor.matmul(psa, lhsT=wk_sb[:, dc * D + ec * P:dc * D + ec * P + P],
                                 rhs=xsT_all[:, dc * XSW:dc * XSW + 512],
                                 start=(dc == 0), stop=(dc == ND - 1))
            for dc in range(ND):
                nc.tensor.matmul(psb, lhsT=wk_sb[:, dc * D + ec * P:dc * D + ec * P + P],
                                 rhs=xsT_all[:, dc * XSW + 512:dc * XSW + 1024],
                                 start=(dc == 0), stop=(dc == ND - 1))
            KTL = 1062                     # last col any window reads
            pst = psA.tile([P, KTL - 1024], f32, tag="psA", name="ps_kt")
            for dc in range(ND):
                nc.tensor.matmul(pst,
                                 lhsT=wk_sb[:, dc * D + ec * P:dc * D + ec * P + P],
                                 rhs=xsT_all[:, dc * XSW + 1024:dc * XSW + KTL],
                                 start=(dc == 0), stop=(dc == ND - 1))
            nc.scalar.activation(out=KT[:, ec * KW:ec * KW + 512], in_=psa,
                                 func=Act.Copy)
            nc.vector.tensor_copy(KT[:, ec * KW + 512:ec * KW + 1024], psb)
            nc.scalar.activation(out=KT[:, ec * KW + 1024:ec * KW + KTL], in_=pst,
                                 func=Act.Copy)

        wf_sb = wpool.tile([P, ND * D], bf16, tag="w", name="wf")
        nc.gpsimd.dma_start(
            out=wf_sb.rearrange("p (b c) -> p b c", b=ND),
            in_=dram_blocks_ap(wf, ND, D),
        )
        xq_all = big.tile([P, NT * D], bf16, tag="xq")
        nc.gpsimd.dma_start(
            out=xq_all.rearrange("p (b c) -> p b c", b=NT),
            in_=dram_blocks_ap(xq, NT, D),
        )

        # ---- V projection helper: window-aligned chunk tiles (halo rows),
        # emitted interleaved with attention to keep the PE warm ----
        def emit_vproj(ci):
            s = CL * ci
            psa = psA.tile([P, 512], f32, tag="psA", name="ps_va")
            psb = psA.tile([P, 512], f32, tag="psA", name="ps_vb")
            for dc in range(ND):
                nc.tensor.matmul(psa, lhsT=xsT_all[:, dc * XSW + s:dc * XSW + s + P],
                                 rhs=wv_sb[:, dc * D:dc * D + 512],
                                 start=(dc == 0), stop=(dc == ND - 1))
            for dc in range(ND):
                nc.tensor.matmul(psb, lhsT=xsT_all[:, dc * XSW + s:dc * XSW + s + P],
                                 rhs=wv_sb[:, dc * D + 512:dc * D + 1024],
                                 start=(dc == 0), stop=(dc == ND - 1))
            nc.scalar.activation(out=V_all[:, ci * D:ci * D + 512], in_=psa,
                                 func=Act.Copy)
            nc.vector.tensor_copy(V_all[:, ci * D + 512:ci * D + 1024], psb)

        # ---- attention + FC, software-pipelined ----
        # FC chunk c is emitted once PV of its source chunks is emitted.
        fc_at = {2: [0], 3: [1], 4: [2], 6: [3], 7: [4], 8: [5], 10: [6], 11: [7]}

        # slot sl of a group holds head 8g + perm(sl); slots 0-3 (psum bank 0)
        # take the partition-base-0 heads, slots 4-7 (bank 1) the base-64
        # heads: consecutive matmuls into one PSUM bank must share the PE
        # tile row (lhsT partition base) or the exec unit dies.
        def s_head(g, sl):
            return 8 * g + (sl % 4) * 2 + sl // 4

        def emit_scores(ci, g):
            s = CL * ci
            s2 = psS.tile([CL, 1024], f32, tag="psS", name=f"s2_{ci}_{g}")
            for sl in range(8):
                ec = 4 * g + sl % 4
                r = sl // 4
                nc.tensor.matmul(
                    s2[:, sl * P:sl * P + W],
                    lhsT=QT[64 * r:64 * r + 64, ec * TQ + s:ec * TQ + s + CL],
                    rhs=KT[64 * r:64 * r + 64, ec * KW + s:ec * KW + s + W],
                    start=True, stop=True,
                )
            return s2

        def emit_exp(ci, g, s2, pe):
            # exp of one group's scores into its half of the shared pe tile
            pev = pe.rearrange("p (h c) -> p h c", h=16)[:, 8 * g:8 * g + 8, 0:W]
            nc.scalar.activation(
                out=pev,
                in_=s2.rearrange("p (h c) -> p h c", h=8)[:, :, 0:W],
                func=Act.Exp)

        def emit_softmax_tail(ci, pe):
            # band-mask, denominators, normalize: all 16 heads in one op each
            pev = pe.rearrange("p (h c) -> p h c", h=16)[:, :, 0:W]
            nc.vector.tensor_tensor(
                pev, pev,
                band_sb[:, None, 0:W].to_broadcast((CL, 16, W)),
                Alu.mult,
            )
            den = small.tile([CL, 16], f32, tag="den", name="den")
            nc.vector.tensor_reduce(out=den, in_=pev,
                                    axis=mybir.AxisListType.X, op=Alu.add)
            rcp = small.tile([CL, 16], f32, tag="rcp", name="rcp")
            nc.vector.reciprocal(rcp, den)
            pn = pnpool.tile([CL, 2048], bf16, tag="pn", name=f"pn_{ci}")
            nc.vector.tensor_tensor(
                pn.rearrange("p (h c) -> p h c", h=16)[:, :, 0:W],
                pev,
                rcp[:, :, None].to_broadcast((CL, 16, W)),
                Alu.mult,
            )
            return pn

        def emit_transposes(ci, g, pn):
            pt = psX.tile([P, 512], f32, tag="psX",
                          name=f"pt_{ci}_{g}").bitcast(bf16)
            for h in range(8):
                nc.tensor.transpose(pt[:, h * P:h * P + CL],
                                    pn[:, (8 * g + h) * P:(8 * g + h) * P + P],
                                    idn_sb[0:CL, 0:CL])
            ptsb = ptpool.tile([P, 1024], bf16, tag="pt", name=f"ptsb_{ci}_{g}")
            src = pt.rearrange("p (h c) -> p h c", h=8)[:, :, 0:CL]
            dst = ptsb.rearrange("p (h c) -> p h c", h=8)[:, :, 0:CL]
            if g == 0:
                nc.vector.tensor_copy(dst, src)
            else:
                nc.scalar.activation(out=dst, in_=src, func=Act.Copy)
            return ptsb

        def emit_pv(ci, g, ptsb):
            ot = psX.tile([P, 512], f32, tag="psX", name=f"ot_{ci}_{g}")
            for sl in range(8):
                hh = s_head(g, sl)
                hl = hh - 8 * g
                p_, r = hl // 2, hl % 2
                nc.tensor.matmul(
                    ot[64 * r:64 * r + 64, p_ * P:p_ * P + CL],
                    lhsT=V_all[0:W, ci * D + hh * DV:ci * D + hh * DV + DV],
                    rhs=ptsb[0:W, sl * P:sl * P + CL],
                    start=True, stop=True,
                )
            # evict: pair p of this group -> OT block (4g + p), token cols
            s = CL * ci
            ew = CL if ci < NCH - 1 else T - s   # last chunk: only 64 valid
            otv = OT.rearrange("p (b c) -> p b c", b=ND)
            nc.scalar.activation(
                out=otv[:, 4 * g:4 * g + 4, s:s + ew],
                in_=ot.rearrange("p (h c) -> p h c", h=4)[:, :, 0:ew],
                func=Act.Copy,
            )

        def emit_fc(c):
            cs = c * P
            ps = psF.tile([P, 1024], f32, tag="psF", name=f"ps_f{c}")
            for half in (0, 1):
                hs = 512 * half
                for ec in range(ND):
                    nc.tensor.matmul(ps[:, hs:hs + 512],
                                     lhsT=OT[:, ec * T + cs:ec * T + cs + P],
                                     rhs=wf_sb[:, ec * D + hs:ec * D + hs + 512],
                                     start=(ec == 0), stop=(ec == ND - 1))
            # evict y = fc + residual to SBUF; frees psF for the next chunk
            yr = ypool.tile([P, D], f32, tag="yr", name=f"yr{c}")
            nc.vector.tensor_add(yr, ps, xq_all[:, c * D:c * D + 1024])
            ysum = lns.tile([P, 1], f32, tag="ysum", name="ysum")
            nc.vector.tensor_reduce(out=ysum, in_=yr,
                                    axis=mybir.AxisListType.X, op=Alu.add)
            ssum = lns.tile([P, 1], f32, tag="ssum", name="ssum")
            ysq = lns.tile([P, 1024], f32, tag="ysq", name="ysq")
            nc.scalar.activation(out=ysq, in_=yr, func=Act.Square, accum_out=ssum)
            mean = lns.tile([P, 1], f32, tag="mean", name="mean")
            nc.vector.tensor_scalar_mul(mean, ysum, -1.0 / D)   # negated mean
            msq = lns.tile([P, 1], f32, tag="msq", name="msq")
            nc.vector.tensor_mul(msq, mean, mean)
            var = lns.tile([P, 1], f32, tag="var", name="var")
            nc.vector.tensor_scalar_mul(var, ssum, 1.0 / D)
            nc.vector.tensor_sub(var, var, msq)
            nc.vector.tensor_add(var, var, eps_sb)
            # rstd = 1/sqrt(var) on gpsimd: magic-seed + 2 Newton steps
            # (avoids ACT sqrt/ln -> no activation-table reloads)
            i32 = mybir.dt.int32
            sh = lns.tile([P, 1], i32, tag="sh", name="sh")
            nc.vector.tensor_scalar(out=sh, in0=var.bitcast(i32), scalar1=1,
                                    scalar2=None, op0=Alu.arith_shift_right)
            nt = lns.tile([P, 1], i32, tag="nt", name="nt")
            nc.vector.tensor_tensor(nt, sh, sh, Alu.bitwise_not)
            sd = lns.tile([P, 1], i32, tag="sd", name="sd")
            # magic - (i>>1) == not(i>>1) - not(magic)
            nc.vector.tensor_scalar(out=sd, in0=nt, scalar1=~0x5f3759df,
                                    scalar2=None, op0=Alu.subtract)
            rstd = sd.bitcast(f32)
            t_ = lns.tile([P, 1], f32, tag="t_", name="t_")
            u_ = lns.tile([P, 1], f32, tag="u_", name="u_")
            for _ in range(2):
                nc.vector.tensor_tensor(t_, rstd, rstd, Alu.mult)
                nc.vector.tensor_tensor(t_, t_, var, Alu.mult)
                nc.vector.tensor_scalar(out=u_, in0=t_, scalar1=-0.5,
                                        scalar2=1.5, op0=Alu.mult, op1=Alu.add)
                nc.vector.tensor_tensor(rstd, rstd, u_, Alu.mult)
            bact = lns.tile([P, 1], f32, tag="bact", name="bact")
            nc.vector.tensor_mul(bact, mean, rstd)   # mean already negated
            y = ypool.tile([P, D], f32, tag="y", name=f"y{c}")
            nc.scalar.activation(out=y, in_=yr, func=Act.Identity,
                                 bias=bact, scale=rstd)
            if apply_affine:
                nc.vector.tensor_mul(y, y, g_bc)
                nc.vector.tensor_add(y, y, b_bc)
            nc.sync.dma_start(out=yo[cs:cs + P, :], in_=y)

        emit_vproj(0)
        emit_vproj(1)
        pn_prev = None
        for ci in range(13):
            if ci < NCH:
                s2a = emit_scores(ci, 0)
                pe = pepool.tile([CL, 2048], bf16, tag="pe", name=f"pe_{ci}")
                emit_exp(ci, 0, s2a, pe)
            if 1 <= ci <= NCH:
                pta = emit_transposes(ci - 1, 0, pn_prev)
            if ci < NCH:
                s2b = emit_scores(ci, 1)
                emit_exp(ci, 1, s2b, pe)
            if 1 <= ci <= NCH:
                ptb = emit_transposes(ci - 1, 1, pn_prev)
            if ci + 2 < NCH:
                emit_vproj(ci + 2)
            if 1 <= ci <= NCH:
                emit_pv(ci - 1, 0, pta)
                emit_pv(ci - 1, 1, ptb)
            if ci < NCH:
                pn_prev = emit_softmax_tail(ci, pe)
            if ci >= NCH:
                # keep the PE warm through the drain gaps (HAM re-throttles
                # after idle, making the last FC run at half clock otherwise)
                warm = psA.tile([P, 512], f32, tag="psA", name=f"warm{ci}")
                for _ in range(6):
                    nc.tensor.matmul(warm, lhsT=wf_sb[:, 0:P],
                                     rhs=wf_sb[:, 0:512], start=True, stop=True)
            for c in fc_at.get(ci, []):
                emit_fc(c)

    nc.compile()
    return nc


def _get_program(apply_affine: bool):
    key = ("prog", apply_affine)
    if key not in _CACHE:
        _CACHE[key] = _build_program(apply_affine)
    return _CACHE[key]


def _host_prep(inputs):
    x = np.asarray(inputs["x"], np.float32)
    xs = np.asarray(inputs["xs"], np.float32)
    w_qs = np.asarray(inputs["w_qs"], np.float32)
    b_qs = np.asarray(inputs["b_qs"], np.float32)
    w_ks = np.asarray(inputs["w_ks"], np.float32)
    w_vs = np.asarray(inputs["w_vs"], np.float32)
    b_vs = np.asarray(inputs["b_vs"], np.float32)
    w_fc = np.asarray(inputs["w_fc"], np.float32)
    b_fc = np.asarray(inputs["b_fc"], np.float32)
    ln_g = np.asarray(inputs["ln_g"], np.float32)
    ln_b = np.asarray(inputs["ln_b"], np.float32)

    apply_affine = not (np.all(ln_g == 1.0) and np.all(ln_b == 0.0))

    bprime = (b_vs @ w_fc + b_fc).astype(np.float32)

    band = np.zeros((CL, P), np.float32)
    for t in range(CL):
        band[t, t:t + 2 * NEI + 1] = 1.0

    shared = {
        "wq": np.ascontiguousarray(
            w_qs.reshape(ND, P, ND, P).transpose(2, 0, 1, 3)
                .reshape(ND * D, P).astype(BF16)),
        "wk": np.ascontiguousarray(w_ks.astype(BF16)),
        "wv": np.ascontiguousarray(w_vs.astype(BF16)),
        "wf": np.ascontiguousarray(w_fc.astype(BF16)),
        "bq": np.ascontiguousarray(
            (b_qs / TEMP).reshape(ND, P).T.astype(np.float32)),
        "band": np.ascontiguousarray(band.astype(BF16)),
        "idn": np.eye(P, dtype=BF16),
    }
    if apply_affine:
        shared["lng"] = np.ascontiguousarray(ln_g.reshape(1, D))
        shared["lnb"] = np.ascontiguousarray(ln_b.reshape(1, D))

    in_maps = []
    half_n = S // 2  # 1024
    for core in range(NCORES):
        b, half = core // 2, core % 2
        t0 = half * half_n
        xc = x[b, t0:t0 + half_n]
        halo = np.zeros((TH, D), np.float32)
        lo = max(0, t0 - NEI)
        hi = min(S, t0 + half_n + NEI)
        halo[lo - (t0 - NEI):hi - (t0 - NEI)] = xs[b, lo:hi]
        m = dict(shared)
        m["xq"] = np.ascontiguousarray((xc + bprime).astype(BF16))
        m["xqT"] = np.ascontiguousarray(xc.T.astype(BF16))
        m["xsT"] = np.ascontiguousarray(halo.T.astype(BF16))
        in_maps.append(m)
    return in_maps, apply_affine


def _run(inputs, trace=False, trace_kwargs=None):
    from concourse.bass_utils import run_bass_kernel_spmd

    in_maps, apply_affine = _host_prep(inputs)
    nc = _get_program(apply_affine)
    res = run_bass_kernel_spmd(
        nc, in_maps, list(range(NCORES)),
        trace=trace, **(trace_kwargs or {})
    )
    y = np.empty((B, S, D), np.float32)
    half_n = S // 2
    for core in range(NCORES):
        b, half = core // 2, core % 2
        y[b, half * half_n:(half + 1) * half_n] = res.results[core]["yo"]
    return y, res


def kernel(**inputs):
    y, _ = _run(inputs)
    return y


# revision 22
# speedup vs baseline: 1.0683x; 1.0683x over previous
"""LocalSelfAttention (window=7) Trainium2 Bass kernel, v2.

Full inputs in, full output out. Sharding: 8 cores = batch(4) x seq-half(2),
each core handles 1024 tokens with a 3-token zero-padded halo on xs.

Math rewrites (all exact or bf16-rounding-only):
- K bias drops (softmax shift invariance, incl. zero-padded halo taps).
- V bias + FC bias fold into the residual on the host:
  xq' = x + (b_vs @ w_fc + b_fc); residual enters the FC psum via an
  identity matmul so no DVE copy/add is needed.
- 1/sqrt(dk) folds into the Q projection eviction (ACT scale).
- Band masking is a 0/1 multiply AFTER exp (garbage scores are bounded,
  exp stays finite), so scores go straight from PSUM through one wide
  ACT exp into bf16 SBUF per 8-head group.

Attention: 11 chunks of 96 queries, 102-wide key windows. Scores for 8
heads share one 2-bank PSUM tile (head slots at 128-f32 stride). Softmax
is 3 wide DVE ops per group (band-mult, reduce, normalize-mult) plus a
tiny reciprocal. Per-head PE transposes (identity matmul) and single
PV matmuls (V is projected as 11 window-aligned chunk tiles).

PSUM budget (8 banks): scores 2 + transpose-out 2 + PV-out 2 + proj/FC 2.
"""

import sys

for _p in ("/opt/trn_rl_repo",):
    if _p not in sys.path:
        sys.path.insert(0, _p)

import numpy as np
import ml_dtypes

BF16 = ml_dtypes.bfloat16

H, DK, DV, D = 16, 64, 64, 1024
NEI = 3
TEMP = 8.0
EPS = 1e-5
B, S = 4, 2048
NCORES = 8
T = (B * S) // NCORES          # 1024 tokens per core
TH = T + 2 * NEI               # 1030 halo tokens
P = 128
NT = T // P                    # 8 fc-phase token chunks
ND = D // P                    # 8 feature chunks
CL = 96                        # attention chunk length
NCH = 11                       # attention chunks
TQ = 1056                      # padded query width (NCH*CL)
KW = 1088                      # padded key block width (windows read 128)
XSW = 1152                     # padded xsT block width (V window reads)
W = 102                        # key window (CL + 2*NEI); last chunk: 96

_CACHE = {}


def _build_program(apply_affine: bool):
    import concourse.bacc as bacc
    import concourse.tile as tile
    import concourse.bass as bass
    from concourse import mybir
    from contextlib import ExitStack

    f32 = mybir.dt.float32
    bf16 = mybir.dt.bfloat16
    Alu = mybir.AluOpType
    Act = mybir.ActivationFunctionType

    from concourse import hw_specs as _hw
    _orig_tabs = _hw.get_activation_tables
    def _cov_first(arch, _o=_orig_tabs):
        t = dict(_o(arch))
        key = "natural_log_exp_and_others"
        if key in t:
            t = {key: t[key], **{k: v for k, v in t.items() if k != key}}
        return t
    _cov_first.cache_clear = getattr(_orig_tabs, "cache_clear", lambda: None)
    import os as _os
    if _os.environ.get("ACT_TABLE_PATCH", "0") == "1":
        bacc.get_activation_tables = _cov_first

    nc = bacc.Bacc(
        "TRN2", target_bir_lowering=False, debug=False, enable_asserts=False
    )

    def din(name, shape, dt_):
        return nc.dram_tensor(name, shape, dt_, kind="ExternalInput").ap()

    xqT = din("xqT", (D, T), bf16)        # x^T (host-transposed)
    xsT = din("xsT", (D, TH), bf16)       # xs^T halo (host-transposed)
    xq = din("xq", (T, D), bf16)          # x + bprime, token-major
    wq = din("wq", (D, D), bf16)
    wk = din("wk", (D, D), bf16)
    wv = din("wv", (D, D), bf16)
    wf = din("wf", (D, D), bf16)
    bq = din("bq", (P, ND), f32)          # b_qs/TEMP laid out [p, ec]
    band = din("band", (CL, P), bf16)     # 0/1 band mask (cols>=102 zero)
    idn = din("idn", (P, P), bf16)        # identity for PE transpose/residual
    if apply_affine:
        lng = din("lng", (1, D), f32)
        lnb = din("lnb", (1, D), f32)
    yo = nc.dram_tensor("yo", (T, D), f32, kind="ExternalOutput").ap()

    def dram_blocks_ap(src, nblk, blk_w, valid_w=None, b0=0):
        """AP over src (R, C) viewed as [128 par, nblk, valid_w] where
        block b0+b, partition p reads src row 128*(b0+b) + p, cols 0:valid_w."""
        vw = valid_w if valid_w is not None else blk_w
        c = src.ap[-1][0]  # innermost stride (elements)
        rstride = src.ap[0][0]
        return bass.AP(
            tensor=src.tensor, offset=src.offset + rstride * P * b0,
            ap=[[rstride, P], [rstride * P, nblk], [c, vw]],
        )

    with tile.TileContext(nc) as tc, ExitStack() as ctx:
        consts = ctx.enter_context(tc.tile_pool(name="consts", bufs=1))
        big = ctx.enter_context(tc.tile_pool(name="big", bufs=1))
        wpool = ctx.enter_context(tc.tile_pool(name="wpool", bufs=2))
        pepool = ctx.enter_context(tc.tile_pool(name="pepool", bufs=2))
        pnpool = ctx.enter_context(tc.tile_pool(name="pnpool", bufs=2))
        ptpool = ctx.enter_context(tc.tile_pool(name="ptpool", bufs=2))
        ypool = ctx.enter_context(tc.tile_pool(name="ypool", bufs=2))
        small = ctx.enter_context(tc.tile_pool(name="small", bufs=3))
        lns = ctx.enter_context(tc.tile_pool(name="lns", bufs=2))
        psA = ctx.enter_context(tc.tile_pool(name="psA", bufs=2, space="PSUM"))
        psS = ctx.enter_context(tc.tile_pool(name="psS", bufs=1, space="PSUM"))
        psX = ctx.enter_context(tc.tile_pool(name="psX", bufs=2, space="PSUM"))
        psF = ctx.enter_context(tc.tile_pool(name="psF", bufs=1, space="PSUM"))

        # ---- constants (vector queue; sync starts the big loads) ----
        idn_sb = consts.tile([P, P], bf16, tag="idn")
        nc.gpsimd.dma_start(out=idn_sb, in_=idn)
        band_sb = consts.tile([CL, P], bf16, tag="band")
        nc.gpsimd.dma_start(out=band_sb, in_=band)
        bq_sb = consts.tile([P, ND], f32, tag="bq")
        nc.gpsimd.dma_start(out=bq_sb, in_=bq)
        # Q-proj inputs first: xT halves + wq halves on sync/scalar
        eps_sb = consts.tile([P, 1], f32, tag="eps")
        nc.vector.memset(eps_sb, EPS)
        if apply_affine:
            g_bc = consts.tile([P, D], f32, tag="g_bc")
            b_bc = consts.tile([P, D], f32, tag="b_bc")
            nc.sync.dma_start(
                out=g_bc,
                in_=bass.AP(tensor=lng.tensor, offset=lng.offset,
                            ap=[[0, P]] + list(lng.ap[1:])),
            )
            nc.sync.dma_start(
                out=b_bc,
                in_=bass.AP(tensor=lnb.tensor, offset=lnb.offset,
                            ap=[[0, P]] + list(lnb.ap[1:])),
            )

        # ---- big loads: Q-proj inputs first, split fine to engage many
        # DMA engines (each dma_start lands on its own engine) ----
        qs = [nc.sync, nc.scalar, nc.gpsimd]
        wq_sb = wpool.tile([P, ND * D], bf16, tag="w", name="wq")
        wqv = wq_sb.rearrange("p (b c) -> p b c", b=ND)
        xT_all = big.tile([P, ND * T], bf16, tag="xT")
        xtv = xT_all.rearrange("p (b c) -> p b c", b=ND)
        for i in range(4):
            qs[i % 3].dma_start(out=wqv[:, 2 * i:2 * i + 2, :],
                                in_=dram_blocks_ap(wq, 2, D, b0=2 * i))
            qs[(i + 1) % 3].dma_start(out=xtv[:, 2 * i:2 * i + 2, :],
                                      in_=dram_blocks_ap(xqT, 2, T, b0=2 * i))
        xsT_all = big.tile([P, ND * XSW], bf16, tag="xsT")
        xsv = xsT_all.rearrange("p (b c) -> p b c", b=ND)
        wk_sb = wpool.tile([P, ND * D], bf16, tag="w", name="wk")
        wkv = wk_sb.rearrange("p (b c) -> p b c", b=ND)
        for i in range(4):
            qs[i % 3].dma_start(out=xsv[:, 2 * i:2 * i + 2, 0:TH],
                                in_=dram_blocks_ap(xsT, 2, XSW, TH, b0=2 * i))
            qs[(i + 1) % 3].dma_start(out=wkv[:, 2 * i:2 * i + 2, :],
                                      in_=dram_blocks_ap(wk, 2, D, b0=2 * i))
        nc.vector.memset(xsv[:, :, TH:XSW], 0.0)

        QT = big.tile([P, ND * TQ], bf16, tag="QT")
        KT = big.tile([P, ND * KW], bf16, tag="KT")
        V_all = big.tile([P, NCH * D], bf16, tag="V")
        OT = big.tile([P, ND * T], bf16, tag="OT")

        # ---- Q projection: feature-major, bias+1/TEMP via ACT evict ----
        for ec in range(ND):
            psa = psA.tile([P, 512], f32, tag="psA", name="ps_qa")
            psb = psA.tile([P, 512], f32, tag="psA", name="ps_qb")
            for dc in range(ND):
                nc.tensor.matmul(psa, lhsT=wq_sb[:, dc * D + ec * P:dc * D + ec * P + P],
                                 rhs=xT_all[:, dc * T:dc * T + 512],
                                 start=(dc == 0), stop=(dc == ND - 1))
            for dc in range(ND):
                nc.tensor.matmul(psb, lhsT=wq_sb[:, dc * D + ec * P:dc * D + ec * P + P],
                                 rhs=xT_all[:, dc * T + 512:dc * T + 1024],
                                 start=(dc == 0), stop=(dc == ND - 1))
            nc.scalar.activation(out=QT[:, ec * TQ:ec * TQ + 512], in_=psa,
                                 func=Act.Identity,
                                 bias=bq_sb[:, ec:ec + 1], scale=1.0 / TEMP)
            nc.vector.scalar_tensor_tensor(
                out=QT[:, ec * TQ + 512:ec * TQ + 1024], in0=psb,
                scalar=1.0 / TEMP,
                in1=bq_sb[:, ec:ec + 1].to_broadcast((P, 512)),
                op0=Alu.mult, op1=Alu.add,
            )
        # zero the 32 pad query columns of each block
        qv = QT.rearrange("p (b c) -> p b c", b=ND)
        nc.vector.memset(qv[:, :, T:TQ], 0.0)

        wv_sb = wpool.tile([P, ND * D], bf16, tag="w", name="wv")
        nc.gpsimd.dma_start(
            out=wv_sb.rearrange("p (b c) -> p b c", b=ND),
            in_=dram_blocks_ap(wv, ND, D),
        )

        # ---- K projection (no bias); KT cols beyond 1030 are 0 ----
        for ec in range(ND):
            psa = psA.tile([P, 512], f32, tag="psA", name="ps_ka")
            psb = psA.tile([P, 512], f32, tag="psA", name="ps_kb")
            for dc in range(ND):
                nc.tensor.matmul(psa, lhsT=wk_sb[:, dc * D + ec * P:dc * D + ec * P + P],
                                 rhs=xsT_all[:, dc * XSW:dc * XSW + 512],
                                 start=(dc == 0), stop=(dc == ND - 1))
            for dc in range(ND):
                nc.tensor.matmul(psb, lhsT=wk_sb[:, dc * D + ec * P:dc * D + ec * P + P],
                                 rhs=xsT_all[:, dc * XSW + 512:dc * XSW + 1024],
                                 start=(dc == 0), stop=(dc == ND - 1))
            KTL = 1062                     # last col any window reads
            pst = psA.tile([P, KTL - 1024], f32, tag="psA", name="ps_kt")
            for dc in range(ND):
                nc.tensor.matmul(pst,
                                 lhsT=wk_sb[:, dc * D + ec * P:dc * D + ec * P + P],
                                 rhs=xsT_all[:, dc * XSW + 1024:dc * XSW + KTL],
                                 start=(dc == 0), stop=(dc == ND - 1))
            nc.scalar.activation(out=KT[:, ec * KW:ec * KW + 512], in_=psa,
                                 func=Act.Copy)
            nc.vector.tensor_copy(KT[:, ec * KW + 512:ec * KW + 1024], psb)
            nc.scalar.activation(out=KT[:, ec * KW + 1024:ec * KW + KTL], in_=pst,
                                 func=Act.Copy)

        wf_sb = wpool.tile([P, ND * D], bf16, tag="w", name="wf")
        nc.gpsimd.dma_start(
            out=wf_sb.rearrange("p (b c) -> p b c", b=ND),
            in_=dram_blocks_ap(wf, ND, D),
        )
        xq_all = big.tile([P, NT * D], bf16, tag="xq")
        nc.gpsimd.dma_start(
            out=xq_all.rearrange("p (b c) -> p b c", b=NT),
            in_=dram_blocks_ap(xq, NT, D),
        )

        # ---- V projection helper: window-aligned chunk tiles (halo rows),
        # emitted interleaved with attention to keep the PE warm ----
        def emit_vproj(ci):
            s = CL * ci
            psa = psA.tile([P, 512], f32, tag="psA", name="ps_va")
            psb = psA.tile([P, 512], f32, tag="psA", name="ps_vb")
            for dc in range(ND):
                nc.tensor.matmul(psa, lhsT=xsT_all[:, dc * XSW + s:dc * XSW + s + P],
                                 rhs=wv_sb[:, dc * D:dc * D + 512],
                                 start=(dc == 0), stop=(dc == ND - 1))
            for dc in range(ND):
                nc.tensor.matmul(psb, lhsT=xsT_all[:, dc * XSW + s:dc * XSW + s + P],
                                 rhs=wv_sb[:, dc * D + 512:dc * D + 1024],
                                 start=(dc == 0), stop=(dc == ND - 1))
            nc.scalar.activation(out=V_all[:, ci * D:ci * D + 512], in_=psa,
                                 func=Act.Copy)
            nc.vector.tensor_copy(V_all[:, ci * D + 512:ci * D + 1024], psb)

        # ---- attention + FC, software-pipelined ----
        # FC chunk c is emitted once PV of its source chunks is emitted.
        fc_at = {2: [0], 3: [1], 4: [2], 6: [3], 7: [4], 8: [5], 10: [6], 11: [7]}

        # slot sl of a group holds head 8g + perm(sl); slots 0-3 (psum bank 0)
        # take the partition-base-0 heads, slots 4-7 (bank 1) the base-64
        # heads: consecutive matmuls into one PSUM bank must share the PE
        # tile row (lhsT partition base) or the exec unit dies.
        def s_head(g, sl):
            return 8 * g + (sl % 4) * 2 + sl // 4

        def emit_scores(ci, g):
            s = CL * ci
            s2 = psS.tile([CL, 1024], f32, tag="psS", name=f"s2_{ci}_{g}")
            for sl in range(8):
                ec = 4 * g + sl % 4
                r = sl // 4
                nc.tensor.matmul(
                    s2[:, sl * P:sl * P + W],
                    lhsT=QT[64 * r:64 * r + 64, ec * TQ + s:ec * TQ + s + CL],
                    rhs=KT[64 * r:64 * r + 64, ec * KW + s:ec * KW + s + W],
                    start=True, stop=True,
                )
            return s2

        def emit_exp(ci, g, s2, pe):
            # exp of one group's scores into its half of the shared pe tile
            pev = pe.rearrange("p (h c) -> p h c", h=16)[:, 8 * g:8 * g + 8, 0:W]
            nc.scalar.activation(
                out=pev,
                in_=s2.rearrange("p (h c) -> p h c", h=8)[:, :, 0:W],
                func=Act.Exp)

        def emit_softmax_tail(ci, pe):
            # band-mask, denominators, normalize: all 16 heads in one op each
            pev = pe.rearrange("p (h c) -> p h c", h=16)[:, :, 0:W]
            nc.vector.tensor_tensor(
                pev, pev,
                band_sb[:, None, 0:W].to_broadcast((CL, 16, W)),
                Alu.mult,
            )
            den = small.tile([CL, 16], f32, tag="den", name="den")
            nc.vector.tensor_reduce(out=den, in_=pev,
                                    axis=mybir.AxisListType.X, op=Alu.add)
            rcp = small.tile([CL, 16], f32, tag="rcp", name="rcp")
            nc.vector.reciprocal(rcp, den)
            pn = pnpool.tile([CL, 2048], bf16, tag="pn", name=f"pn_{ci}")
            nc.vector.tensor_tensor(
                pn.rearrange("p (h c) -> p h c", h=16)[:, :, 0:W],
                pev,
                rcp[:, :, None].to_broadcast((CL, 16, W)),
                Alu.mult,
            )
            return pn

        def emit_transposes(ci, g, pn):
            pt = psX.tile([P, 512], f32, tag="psX",
                          name=f"pt_{ci}_{g}").bitcast(bf16)
            for h in range(8):
                nc.tensor.transpose(pt[:, h * P:h * P + CL],
                                    pn[:, (8 * g + h) * P:(8 * g + h) * P + P],
                                    idn_sb[0:CL, 0:CL])
            ptsb = ptpool.tile([P, 1024], bf16, tag="pt", name=f"ptsb_{ci}_{g}")
            src = pt.rearrange("p (h c) -> p h c", h=8)[:, :, 0:CL]
            dst = ptsb.rearrange("p (h c) -> p h c", h=8)[:, :, 0:CL]
            if g == 0:
                nc.vector.tensor_copy(dst, src)
            else:
                nc.scalar.activation(out=dst, in_=src, func=Act.Copy)
            return ptsb

        def emit_pv(ci, g, ptsb):
            ot = psX.tile([P, 512], f32, tag="psX", name=f"ot_{ci}_{g}")
            for sl in range(8):
                hh = s_head(g, sl)
                hl = hh - 8 * g
                p_, r = hl // 2, hl % 2
                nc.tensor.matmul(
                    ot[64 * r:64 * r + 64, p_ * P:p_ * P + CL],
                    lhsT=V_all[0:W, ci * D + hh * DV:ci * D + hh * DV + DV],
                    rhs=ptsb[0:W, sl * P:sl * P + CL],
                    start=True, stop=True,
                )
            # evict: pair p of this group -> OT block (4g + p), token cols
            s = CL * ci
            ew = CL if ci < NCH - 1 else T - s   # last chunk: only 64 valid
            otv = OT.rearrange("p (b c) -> p b c", b=ND)
            nc.scalar.activation(
                out=otv[:, 4 * g:4 * g + 4, s:s + ew],
                in_=ot.rearrange("p (h c) -> p h c", h=4)[:, :, 0:ew],
                func=Act.Copy,
            )

        def emit_fc(c):
            cs = c * P
            ps = psF.tile([P, 1024], f32, tag="psF", name=f"ps_f{c}")
            for half in (0, 1):
                hs = 512 * half
                for ec in range(ND):
                    nc.tensor.matmul(ps[:, hs:hs + 512],
                                     lhsT=OT[:, ec * T + cs:ec * T + cs + P],
                                     rhs=wf_sb[:, ec * D + hs:ec * D + hs + 512],
                                     start=(ec == 0), stop=(ec == ND - 1))
            # evict y = fc + residual to SBUF; frees psF for the next chunk
            yr = ypool.tile([P, D], f32, tag="yr", name=f"yr{c}")
            nc.vector.tensor_add(yr, ps, xq_all[:, c * D:c * D + 1024])
            ysum = lns.tile([P, 1], f32, tag="ysum", name="ysum")
            nc.vector.tensor_reduce(out=ysum, in_=yr,
                                    axis=mybir.AxisListType.X, op=Alu.add)
            ssum = lns.tile([P, 1], f32, tag="ssum", name="ssum")
            ysq = lns.tile([P, 1024], f32, tag="ysq", name="ysq")
            nc.scalar.activation(out=ysq, in_=yr, func=Act.Square, accum_out=ssum)
            mean = lns.tile([P, 1], f32, tag="mean", name="mean")
            nc.vector.tensor_scalar_mul(mean, ysum, -1.0 / D)   # negated mean
            msq = lns.tile([P, 1], f32, tag="msq", name="msq")
            nc.vector.tensor_mul(msq, mean, mean)
            var = lns.tile([P, 1], f32, tag="var", name="var")
            nc.vector.tensor_scalar_mul(var, ssum, 1.0 / D)
            nc.vector.tensor_sub(var, var, msq)
            nc.vector.tensor_add(var, var, eps_sb)
            # rstd = 1/sqrt(var) on gpsimd: magic-seed + 2 Newton steps
            # (avoids ACT sqrt/ln -> no activation-table reloads)
            i32 = mybir.dt.int32
            sh = lns.tile([P, 1], i32, tag="sh", name="sh")
            nc.vector.tensor_scalar(out=sh, in0=var.bitcast(i32), scalar1=1,
                                    scalar2=None, op0=Alu.arith_shift_right)
            nt = lns.tile([P, 1], i32, tag="nt", name="nt")
            nc.vector.tensor_tensor(nt, sh, sh, Alu.bitwise_not)
            sd = lns.tile([P, 1], i32, tag="sd", name="sd")
            # magic - (i>>1) == not(i>>1) - not(magic)
            nc.vector.tensor_scalar(out=sd, in0=nt, scalar1=~0x5f3759df,
                                    scalar2=None, op0=Alu.subtract)
            rstd = sd.bitcast(f32)
            t_ = lns.tile([P, 1], f32, tag="t_", name="t_")
            u_ = lns.tile([P, 1], f32, tag="u_", name="u_")
            for _ in range(2):
                nc.vector.tensor_tensor(t_, rstd, rstd, Alu.mult)
                nc.vector.tensor_tensor(t_, t_, var, Alu.mult)
                nc.vector.tensor_scalar(out=u_, in0=t_, scalar1=-0.5,
                                        scalar2=1.5, op0=Alu.mult, op1=Alu.add)
                nc.vector.tensor_tensor(rstd, rstd, u_, Alu.mult)
            bact = lns.tile([P, 1], f32, tag="bact", name="bact")
            nc.vector.tensor_mul(bact, mean, rstd)   # mean already negated
            y = ypool.tile([P, D], f32, tag="y", name=f"y{c}")
            nc.scalar.activation(out=y, in_=yr, func=Act.Identity,
                                 bias=bact, scale=rstd)
            if apply_affine:
                nc.vector.tensor_mul(y, y, g_bc)
                nc.vector.tensor_add(y, y, b_bc)
            nc.sync.dma_start(out=yo[cs:cs + P, :], in_=y)

        emit_vproj(0)
        emit_vproj(1)
        pn_prev = None
        for ci in range(13):
            if ci < NCH:
                s2a = emit_scores(ci, 0)
                pe = pepool.tile([CL, 2048], bf16, tag="pe", name=f"pe_{ci}")
                emit_exp(ci, 0, s2a, pe)
            if 1 <= ci <= NCH:
                pta = emit_transposes(ci - 1, 0, pn_prev)
            if ci < NCH:
                s2b = emit_scores(ci, 1)
                emit_exp(ci, 1, s2b, pe)
            if 1 <= ci <= NCH:
                ptb = emit_transposes(ci - 1, 1, pn_prev)
            if ci + 2 < NCH:
                emit_vproj(ci + 2)
            if 1 <= ci <= NCH:
                emit_pv(ci - 1, 0, pta)
                emit_pv(ci - 1, 1, ptb)
            if ci < NCH:
                pn_prev = emit_softmax_tail(ci, pe)
            if ci >= NCH:
                # keep the PE warm through the drain gaps (HAM re-throttles
                # after idle, making the last FC run at half clock otherwise)
                warm = psA.tile([P, 512], f32, tag="psA", name=f"warm{ci}")
                for _ in range(6):
                    nc.tensor.matmul(warm, lhsT=wf_sb[:, 0:P],
                                     rhs=wf_sb[:, 0:512], start=True, stop=True)
            for c in fc_at.get(ci, []):
                emit_fc(c)

    nc.compile()
    return nc


def _get_program(apply_affine: bool):
    key = ("prog", apply_affine)
    if key not in _CACHE:
        _CACHE[key] = _build_program(apply_affine)
    return _CACHE[key]


def _host_prep(inputs):
    x = np.asarray(inputs["x"], np.float32)
    xs = np.asarray(inputs["xs"], np.float32)
    w_qs = np.asarray(inputs["w_qs"], np.float32)
    b_qs = np.asarray(inputs["b_qs"], np.float32)
    w_ks = np.asarray(inputs["w_ks"], np.float32)
    w_vs = np.asarray(inputs["w_vs"], np.float32)
    b_vs = np.asarray(inputs["b_vs"], np.float32)
    w_fc = np.asarray(inputs["w_fc"], np.float32)
    b_fc = np.asarray(inputs["b_fc"], np.float32)
    ln_g = np.asarray(inputs["ln_g"], np.float32)
    ln_b = np.asarray(inputs["ln_b"], np.float32)

    apply_affine = not (np.all(ln_g == 1.0) and np.all(ln_b == 0.0))

    bprime = (b_vs @ w_fc + b_fc).astype(np.float32)

    band = np.zeros((CL, P), np.float32)
    for t in range(CL):
        band[t, t:t + 2 * NEI + 1] = 1.0

    shared = {
        "wq": np.ascontiguousarray(w_qs.astype(BF16)),
        "wk": np.ascontiguousarray(w_ks.astype(BF16)),
        "wv": np.ascontiguousarray(w_vs.astype(BF16)),
        "wf": np.ascontiguousarray(w_fc.astype(BF16)),
        "bq": np.ascontiguousarray(
            (b_qs / TEMP).reshape(ND, P).T.astype(np.float32)),
        "band": np.ascontiguousarray(band.astype(BF16)),
        "idn": np.eye(P, dtype=BF16),
    }
    if apply_affine:
        shared["lng"] = np.ascontiguousarray(ln_g.reshape(1, D))
        shared["lnb"] = np.ascontiguousarray(ln_b.reshape(1, D))

    in_maps = []
    half_n = S // 2  # 1024
    for core in range(NCORES):
        b, half = core // 2, core % 2
        t0 = half * half_n
        xc = x[b, t0:t0 + half_n]
        halo = np.zeros((TH, D), np.float32)
        lo = max(0, t0 - NEI)
        hi = min(S, t0 + half_n + NEI)
        halo[lo - (t0 - NEI):hi - (t0 - NEI)] = xs[b, lo:hi]
        m = dict(shared)
        m["xq"] = np.ascontiguousarray((xc + bprime).astype(BF16))
        m["xqT"] = np.ascontiguousarray(xc.T.astype(BF16))
        m["xsT"] = np.ascontiguousarray(halo.T.astype(BF16))
        in_maps.append(m)
    return in_maps, apply_affine


def _run(inputs, trace=False, trace_kwargs=None):
    from concourse.bass_utils import run_bass_kernel_spmd

    in_maps, apply_affine = _host_prep(inputs)
    nc = _get_program(apply_affine)
    res = run_bass_kernel_spmd(
        nc, in_maps, list(range(NCORES)),
        trace=trace, **(trace_kwargs or {})
    )
    y = np.empty((B, S, D), np.float32)
    half_n = S // 2
    for core in range(NCORES):
        b, half = core // 2, core % 2
        y[b, half * half_n:(half + 1) * half_n] = res.results[core]["yo"]
    return y, res


def kernel(**inputs):
    y, _ = _run(inputs)
    return y
